# revision 18
# baseline (speedup 1.0000x reference)
"""DeltaNet chunked delta-rule kernel for Trainium2 (Bass/Tile), 8-core SPMD.

Full inputs: q,k,v [4,8,4096,128] fp32, beta [4,8,4096] fp32.
Sharding: 32 (b,h) pairs -> 4 per core across 8 cores (state S is per (b,h)).

Algorithm (identical to the CHUNK=32 reference for any chunk size; C=128):
  qh = l2norm(q), kh = l2norm(k), vb = v*beta, kb = kh*beta
  per chunk:  T = kb @ kh^T;  M = I + tril(T,-1);  inv = M^-1
              (exact nilpotent Neumann product inv = prod_j (I + P^(2^j)),
               P = -tril(T,-1))
              u0 = inv @ vb ; w = inv @ kb ; attn = tril(qh kh^T)
  scan:       u = u0 - w @ S ; out = qh @ S + attn @ u ; S += kh^T u

v2 implementation notes (vs the f32r paired baseline):
- All matmuls in bf16 (full PE rate at ANY moving width, so no paired 2x
  compute waste); fp32 PSUM accumulation keeps the scan state exact.
- PSUM drains coalesced: chain results packed 3-to-a-bank, drained with a
  single DVE/ACT copy; drains statically split across Vector and Scalar
  engines to balance load.
- u0 = inv@vb accumulates directly into the scan's u psum tile (no drain);
  negations folded into masks / negated operand copies (kbn, -beta).
"""
import numpy as np

import concourse.bass as bass
import concourse.mybir as mybir
import concourse.tile as tile
from concourse import bacc
from concourse.bass_utils import run_bass_kernel_spmd
from concourse.masks import make_identity, make_lower_triangular, make_upper_triangular

B, H, L, D = 4, 8, 4096, 128
C = 128
NT = L // C
G = 4                 # chunks per load-group
NSEQ = (B * H) // 8   # sequences per core
FP = mybir.dt.float32
BF = mybir.dt.bfloat16
EPS = 1e-6
AF = mybir.ActivationFunctionType
ALU = mybir.AluOpType


def _emit_group_pre(nc, grp, work, cst, dram, s, g):
    """Load q,k,v for 4 chunks; compute rsqrt(norms) for all 4 chunks."""
    q_d, k_d, v_d = dram["q"], dram["k"], dram["v"]
    rows = slice(g * G * C, (g + 1) * G * C)
    rr = lambda ap: ap.rearrange("(gg c) d -> c gg d", gg=G)
    qg = grp.tile([C, G, D], FP, tag="qg", name="qg")
    kg = grp.tile([C, G, D], FP, tag="kg", name="kg")
    vg = grp.tile([C, G, D], FP, tag="vg", name="vg")
    nc.sync.dma_start(out=qg, in_=rr(q_d[s, rows, :]))
    nc.sync.dma_start(out=kg, in_=rr(k_d[s, rows, :]))
    nc.sync.dma_start(out=vg, in_=rr(v_d[s, rows, :]))
    qkss = grp.tile([C, 2 * G], FP, tag="qkss", name="qkss")
    for ci in range(G):
        scr = work.tile([C, D], BF, tag="scr", name="scr")
        scr2 = work.tile([C, D], BF, tag="scr2", name="scr2")
        nc.vector.scalar_tensor_tensor(
            out=scr, in0=qg[:, ci, :], scalar=1.0, in1=qg[:, ci, :],
            op0=ALU.mult, op1=ALU.mult, accum_out=qkss[:, 2 * ci:2 * ci + 1])
        nc.scalar.activation(out=scr2, in_=kg[:, ci, :], func=AF.Square,
                             accum_out=qkss[:, 2 * ci + 1:2 * ci + 2])
    rsq = grp.tile([C, 2 * G], FP, tag="rsq", name="rsq")
    nc.scalar.activation(out=rsq, in_=qkss, func=AF.Sqrt, bias=cst["epsT"][:, 0:1], scale=1.0)
    nc.vector.reciprocal(out=rsq, in_=rsq)
    return qg, kg, vg, rsq


def _emit_chunk(nc, work, pp, pu, cst, S, Sb, bT, bTn, dram, s, i, gtiles):
    qg, kg, vg, rsq = gtiles
    ci = i % G
    o_d = dram["out"]
    rows = slice(i * C, (i + 1) * C)
    identB = cst["identB"]
    rq = rsq[:, 2 * ci:2 * ci + 1]
    rk = rsq[:, 2 * ci + 1:2 * ci + 2]
    bcol = bT[s][:, i:i + 1]
    bncol = bTn[s][:, i:i + 1]

    # ---- preprocessing: qh, kh, kb, kbn (gpsimd), vb (vector) ----
    qh = work.tile([C, D], BF, tag="qh", name="qh")
    kh = work.tile([C, D], BF, tag="kh", name="kh")
    kb = work.tile([C, D], BF, tag="kb", name="kb")
    kbn = work.tile([C, D], BF, tag="kbn", name="kbn")
    vb = work.tile([C, D], BF, tag="vb", name="vb")
    nc.gpsimd.tensor_scalar_mul(qh, qg[:, ci, :], rq)
    nc.gpsimd.tensor_scalar_mul(kh, kg[:, ci, :], rk)
    nc.gpsimd.tensor_scalar(out=kb, in0=kg[:, ci, :], scalar1=rk, scalar2=bcol,
                            op0=ALU.mult, op1=ALU.mult)
    nc.gpsimd.tensor_scalar(out=kbn, in0=kg[:, ci, :], scalar1=rk, scalar2=bncol,
                            op0=ALU.mult, op1=ALU.mult)
    nc.gpsimd.tensor_scalar_mul(vb, vg[:, ci, :], bcol)

    # ---- transposes qT,kT,kbT via DMA xbar (frees PE + DVE + psum) ----
    trs = work.tile([D, 3, C], BF, tag="trs", name="trs")
    nc.sync.dma_start_transpose(trs[:, 0, :], qh)
    nc.sync.dma_start_transpose(trs[:, 1, :], kh)
    nc.sync.dma_start_transpose(trs[:, 2, :], kb)
    qT, kT, kbT = trs[:, 0, :], trs[:, 1, :], trs[:, 2, :]

    # ---- T = kb kh^T + attn (shared bank); P = -tril(T,-1) ----
    ta_ps = pp.tile([C, 2, C], FP, tag="pp", name="ta_ps")
    nc.tensor.matmul(ta_ps[:, 0, :], kbT, kT)
    nc.tensor.matmul(ta_ps[:, 1, :], kT, qT)
    P1 = work.tile([C, C], BF, tag="P1", name="P1")
    nc.vector.tensor_mul(P1, ta_ps[:, 0, :], cst["mSLn"])
    attn_u = work.tile([C, C], BF, tag="attn_u", name="attn_u")
    nc.vector.tensor_copy(attn_u, ta_ps[:, 1, :])
    attnT = work.tile([C, C], BF, tag="attnT", name="attnT")
    nc.gpsimd.tensor_mul(attnT, attn_u, cst["mUIb"])

    # ---- PT1 via DMA transpose ----
    PT1 = work.tile([C, C], BF, tag="PT1", name="PT1")
    nc.sync.dma_start_transpose(PT1, P1)
    R0 = work.tile([C, C], BF, tag="R0", name="R0")
    nc.gpsimd.tensor_add(R0, PT1, identB)

    # ---- Neumann chain: P^(2^j) squarings + R_k = (I + PT_{2^k}) R_{k-1} ----
    # bankA = [P2, PT2]; bankB..D = [P(2e), PT(2e), R]; bankE = [P32,PT32,R4];
    # bankF = [P64, R5]; bankG = [R6] = inv^T
    bankA = pp.tile([C, 2, C], FP, tag="pp", name="bankA")
    nc.tensor.matmul(bankA[:, 0, :], PT1, P1)
    nc.tensor.matmul(bankA[:, 1, :], P1, PT1)
    PPa = work.tile([C, 2, C], BF, tag="PPa", name="PPa")
    nc.vector.tensor_copy(PPa, bankA)
    Pj, PTj = PPa[:, 0, :], PPa[:, 1, :]
    R = R0
    drains = [nc.scalar.copy, nc.scalar.copy, nc.scalar.copy,
              nc.vector.tensor_copy, nc.scalar.copy]
    for j in range(5):  # produces P(4..64) and R1..R5
        nslot = 3 if j < 4 else 2
        bank = pp.tile([C, nslot, C], FP, tag="pp", name=f"bank{j}")
        if j < 4:
            nc.tensor.matmul(bank[:, 0, :], PTj, Pj)      # P(2e)
            nc.tensor.matmul(bank[:, 1, :], Pj, PTj)      # PT(2e)
            nc.tensor.matmul(bank[:, 2, :], Pj, R, start=True, stop=False)
            nc.tensor.matmul(bank[:, 2, :], identB, R, start=False, stop=True)
        else:
            nc.tensor.matmul(bank[:, 0, :], PTj, Pj)      # P64
            nc.tensor.matmul(bank[:, 1, :], Pj, R, start=True, stop=False)
            nc.tensor.matmul(bank[:, 1, :], identB, R, start=False, stop=True)
        nb = work.tile([C, nslot, C], BF, tag=f"nb{j}", name=f"nb{j}")
        drains[j](nb, bank)
        if j < 4:
            Pj, PTj, R = nb[:, 0, :], nb[:, 1, :], nb[:, 2, :]
        else:
            Pj, R = nb[:, 0, :], nb[:, 1, :]
    bankG = pp.tile([C, C], FP, tag="pp", name="bankG")
    nc.tensor.matmul(bankG, Pj, R, start=True, stop=False)
    nc.tensor.matmul(bankG, identB, R, start=False, stop=True)
    invT = work.tile([C, C], BF, tag="invT", name="invT")
    nc.vector.tensor_copy(invT, bankG)

    # ---- scan cluster: one bank [wT, u, out, sd] ----
    sc_ps = pu.tile([C, 4, D], FP, tag="pu", name="sc_ps")
    nc.tensor.matmul(sc_ps[:, 0, :], kbn, invT)                  # -(w^T)
    wTn = work.tile([D, C], BF, tag="wTn", name="wTn")
    nc.vector.tensor_copy(wTn, sc_ps[:, 0, :])
    nc.tensor.matmul(sc_ps[:, 1, :], invT, vb, start=True, stop=False)  # u0

    nc.tensor.matmul(sc_ps[:, 1, :], wTn, Sb[s], start=False, stop=True)  # u = u0 - w@S
    u_bf = work.tile([C, D], BF, tag="u_bf", name="u_bf")
    nc.vector.tensor_copy(u_bf, sc_ps[:, 1, :])
    nc.tensor.matmul(sc_ps[:, 2, :], qT, Sb[s], start=True, stop=False)   # qh @ S
    nc.tensor.matmul(sc_ps[:, 2, :], attnT, u_bf, start=False, stop=True)
    out_sb = work.tile([C, D], FP, tag="out_sb", name="out_sb")
    nc.vector.tensor_copy(out_sb, sc_ps[:, 2, :])
    nc.sync.dma_start(out=o_d[s, rows, :], in_=out_sb)
    nc.tensor.matmul(sc_ps[:, 3, :], kh, u_bf)                    # kh^T u
    nc.vector.tensor_add(S[s], S[s], sc_ps[:, 3, :])
    nc.vector.tensor_copy(Sb[s], S[s])


def build_nc(nseq=NSEQ, nt=NT, repeat=1):
    assert nt % G == 0
    ll = nt * C
    nc = bacc.Bacc(None, target_bir_lowering=False)
    dram = {
        "q": nc.dram_tensor("q", [nseq, ll, D], FP, kind="ExternalInput"),
        "k": nc.dram_tensor("k", [nseq, ll, D], FP, kind="ExternalInput"),
        "v": nc.dram_tensor("v", [nseq, ll, D], FP, kind="ExternalInput"),
        "beta": nc.dram_tensor("beta", [nseq, ll], FP, kind="ExternalInput"),
        "out": nc.dram_tensor("out", [nseq, ll, D], FP, kind="ExternalOutput"),
    }
    with tile.TileContext(nc) as tc:
        with (
            tc.tile_pool(name="consts", bufs=1) as consts,
            tc.tile_pool(name="persist", bufs=1) as persist,
            tc.tile_pool(name="grp", bufs=8) as grp,
            tc.tile_pool(name="work", bufs=6) as work,
            tc.tile_pool(name="pp", bufs=6, space="PSUM") as pp,
            tc.tile_pool(name="pu", bufs=2, space="PSUM") as pu,
        ):
            identF = consts.tile([128, 128], FP, tag="identF", name="identF")
            identB = consts.tile([128, 128], BF, tag="identB", name="identB")
            mSLn = consts.tile([128, 128], FP, tag="mSLn", name="mSLn")
            mUI = consts.tile([128, 128], FP, tag="mUI", name="mUI")
            mUIb = consts.tile([128, 128], BF, tag="mUIb", name="mUIb")
            zeros = consts.tile([128, 128], FP, tag="zeros", name="zeros")
            epsT = consts.tile([128, 1], FP, tag="epsT", name="epsT")
            make_identity(nc, identF)
            nc.vector.tensor_copy(identB, identF)
            make_lower_triangular(nc, mSLn, val=-1.0, diag=False)
            make_upper_triangular(nc, mUI, val=1.0, diag=True)
            nc.vector.tensor_copy(mUIb, mUI)
            nc.gpsimd.memset(zeros, 0.0)
            nc.gpsimd.memset(epsT, EPS)
            cst = dict(identB=identB, mSLn=mSLn, mUI=mUI, mUIb=mUIb, epsT=epsT)

            S, Sb, bT, bTn = [], [], [], []
            for s in range(nseq):
                St = persist.tile([D, D], FP, tag=f"S{s}", name=f"S{s}")
                nc.vector.tensor_copy(St, zeros)
                S.append(St)
                Sbt = persist.tile([D, D], BF, tag=f"Sb{s}", name=f"Sb{s}")
                nc.vector.tensor_copy(Sbt, zeros)
                Sb.append(Sbt)
                bseq = persist.tile([nt, C], FP, tag=f"bseq{s}", name=f"bseq{s}")
                nc.sync.dma_start(out=bseq, in_=dram["beta"][s].rearrange("(n c) -> n c", c=C))
                bt_ps = pp.tile([C, nt], FP, tag="pp", name=f"btps{s}")
                nc.tensor.transpose(bt_ps, bseq, identF[:nt, :nt])
                btile = persist.tile([C, nt], FP, tag=f"bT{s}", name=f"bT{s}")
                nc.vector.tensor_copy(btile, bt_ps)
                bT.append(btile)
                btn = persist.tile([C, nt], FP, tag=f"bTn{s}", name=f"bTn{s}")
                nc.scalar.activation(out=btn, in_=bt_ps, func=AF.Copy, scale=-1.0)
                bTn.append(btn)

            for rep in range(repeat):
                if rep > 0:
                    for s in range(nseq):
                        nc.vector.tensor_copy(S[s], zeros)
                        nc.vector.tensor_copy(Sb[s], zeros)
                for g in range(nt // G):
                    gt = [_emit_group_pre(nc, grp, work, cst, dram, s, g)
                          for s in range(nseq)]
                    for ci in range(G):
                        for s in range(nseq):
                            _emit_chunk(nc, work, pp, pu, cst, S, Sb, bT, bTn,
                                        dram, s, g * G + ci, gt[s])
    nc.compile()
    return nc


_NC_CACHE = None


def _build_in_maps(inputs):
    q = np.ascontiguousarray(np.asarray(inputs["q"], dtype=np.float32))
    k = np.ascontiguousarray(np.asarray(inputs["k"], dtype=np.float32))
    v = np.ascontiguousarray(np.asarray(inputs["v"], dtype=np.float32))
    beta = np.ascontiguousarray(np.asarray(inputs["beta"], dtype=np.float32))
    qf = q.reshape(B * H, L, D)
    kf = k.reshape(B * H, L, D)
    vf = v.reshape(B * H, L, D)
    bf = beta.reshape(B * H, L)
    in_maps = []
    for core in range(8):
        sl = slice(core * NSEQ, (core + 1) * NSEQ)
        in_maps.append({
            "q": np.ascontiguousarray(qf[sl]),
            "k": np.ascontiguousarray(kf[sl]),
            "v": np.ascontiguousarray(vf[sl]),
            "beta": np.ascontiguousarray(bf[sl]),
        })
    return in_maps


def kernel(q, k, v, beta):
    global _NC_CACHE
    if _NC_CACHE is None:
        _NC_CACHE = build_nc()
    nc = _NC_CACHE
    in_maps = _build_in_maps({"q": q, "k": k, "v": v, "beta": beta})
    res = run_bass_kernel_spmd(nc, in_maps, core_ids=list(range(8)))
    out = np.empty((B * H, L, D), dtype=np.float32)
    for core in range(8):
        out[core * NSEQ:(core + 1) * NSEQ] = res.results[core]["out"]
    return out.reshape(B, H, L, D)


# revision 20
# speedup vs baseline: 10.5671x; 10.5671x over previous
"""DeltaNet chunked delta-rule kernel for Trainium2 (Bass/Tile), 8-core SPMD.

Full inputs: q,k,v [4,8,4096,128] fp32, beta [4,8,4096] fp32.
Sharding: 32 (b,h) pairs -> 4 per core across 8 cores (state S is per (b,h)).

Algorithm (identical to the CHUNK=32 reference for any chunk size; C=128):
  qh = l2norm(q), kh = l2norm(k), vb = v*beta, kb = kh*beta
  per chunk:  T = kb @ kh^T;  M = I + tril(T,-1);  inv = M^-1
              (exact nilpotent Neumann product inv = prod_j (I + P^(2^j)),
               P = -tril(T,-1))
              u0 = inv @ vb ; w = inv @ kb ; attn = tril(qh kh^T)
  scan:       u = u0 - w @ S ; out = qh @ S + attn @ u ; S += kh^T u

v2 implementation notes (vs the f32r paired baseline):
- All matmuls in bf16 (full PE rate at ANY moving width, so no paired 2x
  compute waste); fp32 PSUM accumulation keeps the scan state exact.
- PSUM drains coalesced: chain results packed 3-to-a-bank, drained with a
  single DVE/ACT copy; drains statically split across Vector and Scalar
  engines to balance load.
- u0 = inv@vb accumulates directly into the scan's u psum tile (no drain);
  negations folded into masks / negated operand copies (kbn, -beta).
"""
import numpy as np

import concourse.bass as bass
import concourse.mybir as mybir
import concourse.tile as tile
from concourse import bacc
from concourse.bass_utils import run_bass_kernel_spmd
from concourse.masks import make_identity, make_lower_triangular, make_upper_triangular

B, H, L, D = 4, 8, 4096, 128
C = 128
NT = L // C
G = 4                 # chunks per load-group
NSEQ = (B * H) // 8   # sequences per core
FP = mybir.dt.float32
BF = mybir.dt.bfloat16
EPS = 1e-6
AF = mybir.ActivationFunctionType
ALU = mybir.AluOpType


def _emit_group_pre(nc, grp, work, cst, dram, s, g):
    """Load q,k,v for 4 chunks; compute rsqrt(norms) for all 4 chunks."""
    q_d, k_d, v_d = dram["q"], dram["k"], dram["v"]
    rows = slice(g * G * C, (g + 1) * G * C)
    rr = lambda ap: ap.rearrange("(gg c) d -> c gg d", gg=G)
    qg = grp.tile([C, G, D], FP, tag="qg", name="qg")
    kg = grp.tile([C, G, D], FP, tag="kg", name="kg")
    vg = grp.tile([C, G, D], FP, tag="vg", name="vg")
    nc.sync.dma_start(out=qg, in_=rr(q_d[s, rows, :]))
    nc.sync.dma_start(out=kg, in_=rr(k_d[s, rows, :]))
    nc.sync.dma_start(out=vg, in_=rr(v_d[s, rows, :]))
    qkss = grp.tile([C, 2 * G], FP, tag="qkss", name="qkss")
    for ci in range(G):
        scr = work.tile([C, D], BF, tag="scr", name="scr")
        scr2 = work.tile([C, D], BF, tag="scr2", name="scr2")
        nc.vector.scalar_tensor_tensor(
            out=scr, in0=qg[:, ci, :], scalar=1.0, in1=qg[:, ci, :],
            op0=ALU.mult, op1=ALU.mult, accum_out=qkss[:, 2 * ci:2 * ci + 1])
        nc.scalar.activation(out=scr2, in_=kg[:, ci, :], func=AF.Square,
                             accum_out=qkss[:, 2 * ci + 1:2 * ci + 2])
    rsq = grp.tile([C, 2 * G], FP, tag="rsq", name="rsq")
    nc.scalar.activation(out=rsq, in_=qkss, func=AF.Sqrt, bias=cst["epsT"][:, 0:1], scale=1.0)
    nc.vector.reciprocal(out=rsq, in_=rsq)
    return qg, kg, vg, rsq


def _emit_chunk(nc, work, pp, pu, cst, S, Sb, bT, bTn, dram, s, i, gtiles):
    qg, kg, vg, rsq = gtiles
    ci = i % G
    o_d = dram["out"]
    rows = slice(i * C, (i + 1) * C)
    identB = cst["identB"]
    rq = rsq[:, 2 * ci:2 * ci + 1]
    rk = rsq[:, 2 * ci + 1:2 * ci + 2]
    bcol = bT[s][:, i:i + 1]
    bncol = bTn[s][:, i:i + 1]

    # ---- preprocessing: qh, kh, kb, kbn (gpsimd), vb (vector) ----
    qh = work.tile([C, D], BF, tag="qh", name="qh")
    kh = work.tile([C, D], BF, tag="kh", name="kh")
    kb = work.tile([C, D], BF, tag="kb", name="kb")
    kbn = work.tile([C, D], BF, tag="kbn", name="kbn")
    vb = work.tile([C, D], BF, tag="vb", name="vb")
    nc.gpsimd.tensor_scalar_mul(qh, qg[:, ci, :], rq)
    nc.gpsimd.tensor_scalar_mul(kh, kg[:, ci, :], rk)
    nc.gpsimd.tensor_scalar(out=kb, in0=kg[:, ci, :], scalar1=rk, scalar2=bcol,
                            op0=ALU.mult, op1=ALU.mult)
    nc.gpsimd.tensor_scalar(out=kbn, in0=kg[:, ci, :], scalar1=rk, scalar2=bncol,
                            op0=ALU.mult, op1=ALU.mult)
    nc.gpsimd.tensor_scalar_mul(vb, vg[:, ci, :], bcol)

    # ---- transposes qT,kT,kbT: 3 PE transposes into one bank, one drain ----
    # (transpose psum output is bf16 -> 16-bit packed drain, 2x cheaper)
    # merged bank: slots 0-2 = qT,kT,kbT transposes; slot 3 = PT1 (later)
    tr_ps = pp.tile([D, 4, C], BF, tag="ppb", name="tr_ps", bufs=2)
    nc.tensor.matmul(tr_ps[:, 0, :], qh, identB, is_transpose=True)
    nc.tensor.matmul(tr_ps[:, 1, :], kh, identB, is_transpose=True)
    nc.tensor.matmul(tr_ps[:, 2, :], kb, identB, is_transpose=True)
    trs = work.tile([D, 3, C], BF, tag="trs", name="trs")
    nc.vector.tensor_copy(trs, tr_ps[:, 0:3, :])
    qT, kT, kbT = trs[:, 0, :], trs[:, 1, :], trs[:, 2, :]

    # ---- T = kb kh^T + attn (shared bank); P = -tril(T,-1) ----
    ta_ps = pp.tile([C, 2, C], FP, tag="pp", name="ta_ps")
    nc.tensor.matmul(ta_ps[:, 0, :], kbT, kT)
    nc.tensor.matmul(ta_ps[:, 1, :], kT, qT)
    P1 = work.tile([C, C], BF, tag="P1", name="P1")
    nc.vector.tensor_mul(P1, ta_ps[:, 0, :], cst["mSLn"])
    attn_u = work.tile([C, C], BF, tag="attn_u", name="attn_u")
    nc.vector.tensor_copy(attn_u, ta_ps[:, 1, :])
    attnT = work.tile([C, C], BF, tag="attnT", name="attnT")
    nc.gpsimd.tensor_mul(attnT, attn_u, cst["mUIb"])

    # ---- PT1 (PE transpose into slot 3 of the bf16 bank) ----
    nc.tensor.matmul(tr_ps[:, 3, :], P1, identB, is_transpose=True)
    PT1 = work.tile([C, C], BF, tag="PT1", name="PT1")
    nc.vector.tensor_copy(PT1, tr_ps[:, 3, :])
    R0 = work.tile([C, C], BF, tag="R0", name="R0")
    nc.gpsimd.tensor_add(R0, PT1, identB)

    # ---- Neumann chain: P^(2^j) squarings + R_k = (I + PT_{2^k}) R_{k-1} ----
    # bankA = [P2, PT2]; bankB..D = [P(2e), PT(2e), R]; bankE = [P32,PT32,R4];
    # bankF = [P64, R5]; bankG = [R6] = inv^T
    bankA = pp.tile([C, 2, C], FP, tag="pp", name="bankA")
    nc.tensor.matmul(bankA[:, 0, :], PT1, P1)
    nc.tensor.matmul(bankA[:, 1, :], P1, PT1)
    PPa = work.tile([C, 2, C], BF, tag="PPa", name="PPa")
    nc.vector.tensor_copy(PPa, bankA)
    Pj, PTj = PPa[:, 0, :], PPa[:, 1, :]
    R = R0
    drains = [nc.scalar.copy, nc.vector.tensor_copy, nc.scalar.copy,
              nc.vector.tensor_copy, nc.scalar.copy]
    for j in range(5):  # produces P(4..64) and R1..R5
        nslot = 3 if j < 4 else 2
        bank = pp.tile([C, nslot, C], FP, tag="pp", name=f"bank{j}")
        if j < 4:
            nc.tensor.matmul(bank[:, 0, :], PTj, Pj)      # P(2e)
            nc.tensor.matmul(bank[:, 1, :], Pj, PTj)      # PT(2e)
            nc.tensor.matmul(bank[:, 2, :], Pj, R, start=True, stop=False)
            nc.tensor.matmul(bank[:, 2, :], identB, R, start=False, stop=True)
        else:
            nc.tensor.matmul(bank[:, 0, :], PTj, Pj)      # P64
            nc.tensor.matmul(bank[:, 1, :], Pj, R, start=True, stop=False)
            nc.tensor.matmul(bank[:, 1, :], identB, R, start=False, stop=True)
        nb = work.tile([C, nslot, C], BF, tag=f"nb{j}", name=f"nb{j}")
        drains[j](nb, bank)
        if j < 4:
            Pj, PTj, R = nb[:, 0, :], nb[:, 1, :], nb[:, 2, :]
        else:
            Pj, R = nb[:, 0, :], nb[:, 1, :]
    bankG = pp.tile([C, C], FP, tag="pp", name="bankG")
    nc.tensor.matmul(bankG, Pj, R, start=True, stop=False)
    nc.tensor.matmul(bankG, identB, R, start=False, stop=True)
    invT = work.tile([C, C], BF, tag="invT", name="invT")
    nc.vector.tensor_copy(invT, bankG)

    # ---- scan cluster: one bank [wT, u, out, sd] ----
    sc_ps = pu.tile([C, 4, D], FP, tag="pu", name="sc_ps")
    nc.tensor.matmul(sc_ps[:, 0, :], kbn, invT)                  # -(w^T)
    wTn = work.tile([D, C], BF, tag="wTn", name="wTn")
    nc.scalar.copy(wTn, sc_ps[:, 0, :])
    nc.tensor.matmul(sc_ps[:, 1, :], invT, vb, start=True, stop=False)  # u0

    nc.tensor.matmul(sc_ps[:, 1, :], wTn, Sb[s], start=False, stop=True)  # u = u0 - w@S
    u_bf = work.tile([C, D], BF, tag="u_bf", name="u_bf")
    nc.vector.tensor_copy(u_bf, sc_ps[:, 1, :])
    nc.tensor.matmul(sc_ps[:, 2, :], qT, Sb[s], start=True, stop=False)   # qh @ S
    nc.tensor.matmul(sc_ps[:, 2, :], attnT, u_bf, start=False, stop=True)
    out_sb = work.tile([C, D], FP, tag="out_sb", name="out_sb")
    nc.scalar.copy(out_sb, sc_ps[:, 2, :])
    nc.sync.dma_start(out=o_d[s, rows, :], in_=out_sb)
    nc.tensor.matmul(sc_ps[:, 3, :], kh, u_bf)                    # kh^T u
    nc.vector.tensor_add(S[s], S[s], sc_ps[:, 3, :])
    nc.vector.tensor_copy(Sb[s], S[s])


def build_nc(nseq=NSEQ, nt=NT, repeat=1):
    assert nt % G == 0
    ll = nt * C
    nc = bacc.Bacc(None, target_bir_lowering=False)
    dram = {
        "q": nc.dram_tensor("q", [nseq, ll, D], FP, kind="ExternalInput"),
        "k": nc.dram_tensor("k", [nseq, ll, D], FP, kind="ExternalInput"),
        "v": nc.dram_tensor("v", [nseq, ll, D], FP, kind="ExternalInput"),
        "beta": nc.dram_tensor("beta", [nseq, ll], FP, kind="ExternalInput"),
        "out": nc.dram_tensor("out", [nseq, ll, D], FP, kind="ExternalOutput"),
    }
    with tile.TileContext(nc) as tc:
        with (
            tc.tile_pool(name="consts", bufs=1) as consts,
            tc.tile_pool(name="persist", bufs=1) as persist,
            tc.tile_pool(name="grp", bufs=8) as grp,
            tc.tile_pool(name="work", bufs=6) as work,
            tc.tile_pool(name="pp", bufs=4, space="PSUM") as pp,
            tc.tile_pool(name="pu", bufs=2, space="PSUM") as pu,
        ):
            identF = consts.tile([128, 128], FP, tag="identF", name="identF")
            identB = consts.tile([128, 128], BF, tag="identB", name="identB")
            mSLn = consts.tile([128, 128], FP, tag="mSLn", name="mSLn")
            mUI = consts.tile([128, 128], FP, tag="mUI", name="mUI")
            mUIb = consts.tile([128, 128], BF, tag="mUIb", name="mUIb")
            zeros = consts.tile([128, 128], FP, tag="zeros", name="zeros")
            epsT = consts.tile([128, 1], FP, tag="epsT", name="epsT")
            make_identity(nc, identF)
            nc.vector.tensor_copy(identB, identF)
            make_lower_triangular(nc, mSLn, val=-1.0, diag=False)
            make_upper_triangular(nc, mUI, val=1.0, diag=True)
            nc.vector.tensor_copy(mUIb, mUI)
            nc.gpsimd.memset(zeros, 0.0)
            nc.gpsimd.memset(epsT, EPS)
            cst = dict(identB=identB, mSLn=mSLn, mUI=mUI, mUIb=mUIb, epsT=epsT)

            S, Sb, bT, bTn = [], [], [], []
            for s in range(nseq):
                St = persist.tile([D, D], FP, tag=f"S{s}", name=f"S{s}")
                nc.vector.tensor_copy(St, zeros)
                S.append(St)
                Sbt = persist.tile([D, D], BF, tag=f"Sb{s}", name=f"Sb{s}")
                nc.vector.tensor_copy(Sbt, zeros)
                Sb.append(Sbt)
                bseq = persist.tile([nt, C], FP, tag=f"bseq{s}", name=f"bseq{s}")
                nc.sync.dma_start(out=bseq, in_=dram["beta"][s].rearrange("(n c) -> n c", c=C))
                bt_ps = pp.tile([C, nt], FP, tag="pp", name=f"btps{s}")
                nc.tensor.transpose(bt_ps, bseq, identF[:nt, :nt])
                btile = persist.tile([C, nt], FP, tag=f"bT{s}", name=f"bT{s}")
                nc.vector.tensor_copy(btile, bt_ps)
                bT.append(btile)
                btn = persist.tile([C, nt], FP, tag=f"bTn{s}", name=f"bTn{s}")
                nc.scalar.activation(out=btn, in_=bt_ps, func=AF.Copy, scale=-1.0)
                bTn.append(btn)

            for rep in range(repeat):
                if rep > 0:
                    for s in range(nseq):
                        nc.vector.tensor_copy(S[s], zeros)
                        nc.vector.tensor_copy(Sb[s], zeros)
                for g in range(nt // G):
                    gt = [_emit_group_pre(nc, grp, work, cst, dram, s, g)
                          for s in range(nseq)]
                    for ci in range(G):
                        for s in range(nseq):
                            _emit_chunk(nc, work, pp, pu, cst, S, Sb, bT, bTn,
                                        dram, s, g * G + ci, gt[s])
    nc.compile()
    return nc


_NC_CACHE = None


def _build_in_maps(inputs):
    q = np.ascontiguousarray(np.asarray(inputs["q"], dtype=np.float32))
    k = np.ascontiguousarray(np.asarray(inputs["k"], dtype=np.float32))
    v = np.ascontiguousarray(np.asarray(inputs["v"], dtype=np.float32))
    beta = np.ascontiguousarray(np.asarray(inputs["beta"], dtype=np.float32))
    qf = q.reshape(B * H, L, D)
    kf = k.reshape(B * H, L, D)
    vf = v.reshape(B * H, L, D)
    bf = beta.reshape(B * H, L)
    in_maps = []
    for core in range(8):
        sl = slice(core * NSEQ, (core + 1) * NSEQ)
        in_maps.append({
            "q": np.ascontiguousarray(qf[sl]),
            "k": np.ascontiguousarray(kf[sl]),
            "v": np.ascontiguousarray(vf[sl]),
            "beta": np.ascontiguousarray(bf[sl]),
        })
    return in_maps


def kernel(q, k, v, beta):
    global _NC_CACHE
    if _NC_CACHE is None:
        _NC_CACHE = build_nc()
    nc = _NC_CACHE
    in_maps = _build_in_maps({"q": q, "k": k, "v": v, "beta": beta})
    res = run_bass_kernel_spmd(nc, in_maps, core_ids=list(range(8)))
    out = np.empty((B * H, L, D), dtype=np.float32)
    for core in range(8):
        out[core * NSEQ:(core + 1) * NSEQ] = res.results[core]["out"]
    return out.reshape(B, H, L, D)
